# revision 1
# baseline (speedup 1.0000x reference)
"""Fused dequant + residual-add + RMSNorm + int8 requant for TRN2 (8 NeuronCores).

Sharding: tokens (rows) split evenly across the 8 cores; the hidden-dim
reduction stays local and `weight` is replicated.

The quantized GEMM output `x` arrives as int32 but its values fit int16, so
the host narrows it before upload — that cuts the dominant HBM read stream
in half (int16 -> f32 -> *a is bit-identical to int32 -> f32 -> *a for
|x| < 2^24). A range check falls back to the exact int32 kernel.

Per core (2048 x 4096), per 128-row tile:
  ACT : xf = x * a                           (dequant -> f32, in place)
  DVE : res_new = residual + xf              (tensor_tensor add, in place)
  ACT : Square(res_new) with accum_out       (ssq = sum(res_new^2) per row)
  ACT : rms = sqrt(ssq/H + eps);  DVE: rstd = 1/rms
  DVE : yw = res_new * w                     (weight broadcast over partitions)
  ACT : out_i8 = yw * rstd                   (f32 -> int8 is RNE + saturating)

Loads ride the SP HWDGE ring, stores ride gpsimd/SWDGE so a stalled load
never queues ahead of a ready store. The f32->int8 output conversion was
verified on hardware to match np.clip(np.round(x), -128, 127) exactly.

The int16 dequant reads the packed int16 from the TOP half of the f32 tile
and writes f32 over the whole tile in place: the write pointer (4 B/elem
from offset 0) stays strictly behind the read pointer (2 B/elem from half
way), so a streaming elementwise engine never clobbers unread input.
"""

import os

import numpy as np

import concourse.bacc as bacc
import concourse.bass as bass
import concourse.tile as tile
from concourse import mybir
from concourse.bass_utils import run_bass_kernel_spmd

TOKENS = 16384
HIDDEN = 4096
N_CORES = 8
ROWS = TOKENS // N_CORES  # 2048 rows per core
P = 128                   # SBUF partitions
NT = ROWS // P            # 16 row-tiles per core
EPS = 1e-6
SPLIT = 2048              # final-requant column split: ACT [0:SPLIT], DVE [SPLIT:]

_cache: dict = {}
last_results = None  # BassKernelResults of the most recent run (for profiling)


def _build(a: float, x_dtype):
    nc = bacc.Bacc(
        "TRN2", target_bir_lowering=False, debug=False, num_devices=N_CORES
    )
    residual = nc.dram_tensor(
        "residual", [ROWS, HIDDEN], mybir.dt.float32, kind="ExternalInput"
    ).ap()
    x = nc.dram_tensor("x", [ROWS, HIDDEN], x_dtype, kind="ExternalInput").ap()
    weight = nc.dram_tensor(
        "weight", [HIDDEN], mybir.dt.float32, kind="ExternalInput"
    ).ap()
    res_new = nc.dram_tensor(
        "res_new", [ROWS, HIDDEN], mybir.dt.float32, kind="ExternalOutput"
    ).ap()
    out_i8 = nc.dram_tensor(
        "out_i8", [ROWS, HIDDEN], mybir.dt.int8, kind="ExternalOutput"
    ).ap()

    with tile.TileContext(nc) as tc:
        with (
            tc.tile_pool(name="singles", bufs=1) as singles,
            tc.tile_pool(name="work", bufs=4) as work,
            tc.tile_pool(name="sq", bufs=1) as sq_pool,
            tc.tile_pool(name="stats", bufs=4) as stats_pool,
            tc.tile_pool(name="wpsum", bufs=8, space="PSUM") as wpsum,
        ):
            # weight broadcast with zero extra HBM traffic: read the 16 KiB row
            # once (ACT HWDGE ring), then ones[1,128]^T @ w_row[1,512-chunk] on
            # the otherwise-idle PE replicates it across all 128 partitions.
            # K=1 fp32 matmul is exact (bf16-decomposed terms of w times 1.0,
            # accumulated in f32 PSUM).
            w_row = singles.tile([1, HIDDEN], mybir.dt.float32)
            nc.scalar.dma_start(out=w_row[:], in_=weight[None, :])
            ones1 = singles.tile([1, P], mybir.dt.float32)
            nc.vector.memset(ones1[:], 1.0)
            w_b = singles.tile([P, HIDDEN], mybir.dt.float32)
            for j in range(HIDDEN // 512):
                ps = wpsum.tile([P, 512], mybir.dt.float32, tag="wp")
                nc.tensor.matmul(
                    ps[:], ones1[:], w_row[:, j * 512 : (j + 1) * 512],
                    start=True, stop=True,
                )
                nc.scalar.copy(w_b[:, j * 512 : (j + 1) * 512], ps[:])
            eps_t = singles.tile([P, 1], mybir.dt.float32)
            nc.vector.memset(eps_t[:], EPS)
            # scratch for Square's elementwise output (only accum_out is used;
            # ACT executes in order so one buffer never races itself)
            sq = sq_pool.tile([P, HIDDEN], mybir.dt.float32)

            H2 = HIDDEN // 2
            for it in range(NT):
                r0 = it * P
                xi = work.tile([P, HIDDEN], mybir.dt.float32, tag="xi")
                xf = xi[:]
                res = work.tile([P, HIDDEN], mybir.dt.float32, tag="res")
                if it == 0 and x_dtype == mybir.dt.int16:
                    # first tile: split loads + dequant by column half so the
                    # first res_new store issues ~8 us earlier (writes start
                    # while the read-only ramp is still running). The int16
                    # halves stage in the sq scratch (first used by square()
                    # later), so the dequants never alias their own output.
                    sq16 = sq[:].bitcast(mybir.dt.int16)
                    for k, (c0, c1) in enumerate(((0, H2), (H2, HIDDEN))):
                        stage = sq16[:, c0:c1]
                        nc.sync.dma_start(out=stage, in_=x[r0 : r0 + P, c0:c1])
                        nc.sync.dma_start(
                            out=res[:, c0:c1], in_=residual[r0 : r0 + P, c0:c1]
                        )
                        nc.scalar.mul(xf[:, c0:c1], stage, a)
                else:
                    if x_dtype == mybir.dt.int16:
                        xi_in = xi[:].bitcast(mybir.dt.int16)[
                            :, HIDDEN : 2 * HIDDEN
                        ]
                    else:
                        xi_in = xi[:].bitcast(mybir.dt.int32)
                    nc.sync.dma_start(out=xi_in, in_=x[r0 : r0 + P, :])
                    nc.sync.dma_start(out=res[:], in_=residual[r0 : r0 + P, :])
                    nc.scalar.mul(xf, xi_in, a)  # dequant in place

                if 0 < it < NT - 2:
                    nc.vector.tensor_add(res[:], res[:], xf)
                    nc.gpsimd.dma_start(out=res_new[r0 : r0 + P, :], in_=res[:])

                    # ssq = sum(res^2) along hidden; rms = sqrt(ssq/H + eps)
                    ssq = stats_pool.tile([P, 1], mybir.dt.float32, tag="ssq")
                    nc.scalar.activation(
                        sq[:], res[:], mybir.ActivationFunctionType.Square,
                        accum_out=ssq[:],
                    )
                    rms = stats_pool.tile([P, 1], mybir.dt.float32, tag="rms")
                    nc.scalar.activation(
                        rms[:], ssq[:], mybir.ActivationFunctionType.Sqrt,
                        bias=eps_t[:], scale=1.0 / HIDDEN,
                    )
                    rstd = stats_pool.tile([P, 1], mybir.dt.float32, tag="rstd")
                    nc.vector.reciprocal(rstd[:], rms[:])

                    # xi/xf is dead after the add — reuse for res_new * w
                    nc.vector.tensor_mul(xf, res[:], w_b[:])
                    # final requant split across ACT and DVE to balance busy
                    # time (both convert f32 -> int8 with RNE + saturation)
                    o8 = work.tile([P, HIDDEN], mybir.dt.int8, tag="o8")
                    nc.scalar.mul(o8[:, :SPLIT], xf[:, :SPLIT], rstd[:])
                    nc.vector.tensor_scalar_mul(
                        o8[:, SPLIT:], xf[:, SPLIT:], rstd[:]
                    )
                    nc.gpsimd.dma_start(out=out_i8[r0 : r0 + P, :], in_=o8[:])
                else:
                    # first and last tiles: column-halved so the first store
                    # issues earlier and the pipeline drain is half as deep
                    ssq_h = stats_pool.tile([P, 2], mybir.dt.float32, tag="ssqh")
                    last = it == NT - 1
                    for k, (c0, c1) in enumerate(((0, H2), (H2, HIDDEN))):
                        nc.vector.tensor_add(
                            res[:, c0:c1], res[:, c0:c1], xf[:, c0:c1]
                        )
                        # very last res half rides the (by then idle) SP ring
                        res_eng = nc.sync if (last and k == 1) else nc.gpsimd
                        res_eng.dma_start(
                            out=res_new[r0 : r0 + P, c0:c1], in_=res[:, c0:c1]
                        )
                        nc.scalar.activation(
                            sq[:, c0:c1], res[:, c0:c1],
                            mybir.ActivationFunctionType.Square,
                            accum_out=ssq_h[:, k : k + 1],
                        )
                    ssq = stats_pool.tile([P, 1], mybir.dt.float32, tag="ssq")
                    nc.vector.tensor_add(ssq[:], ssq_h[:, 0:1], ssq_h[:, 1:2])
                    rms = stats_pool.tile([P, 1], mybir.dt.float32, tag="rms")
                    nc.scalar.activation(
                        rms[:], ssq[:], mybir.ActivationFunctionType.Sqrt,
                        bias=eps_t[:], scale=1.0 / HIDDEN,
                    )
                    rstd = stats_pool.tile([P, 1], mybir.dt.float32, tag="rstd")
                    nc.vector.reciprocal(rstd[:], rms[:])
                    o8 = work.tile([P, HIDDEN], mybir.dt.int8, tag="o8")
                    nc.vector.tensor_mul(xf[:, 0:H2], res[:, 0:H2], w_b[:, 0:H2])
                    nc.scalar.mul(o8[:, 0:H2], xf[:, 0:H2], rstd[:])
                    # last tile's ACT-produced half follows ACT's own final op
                    # on the ACT HWDGE ring (no wait), spreading the tail drain
                    o8_eng0 = nc.scalar if last else nc.gpsimd
                    o8_eng0.dma_start(
                        out=out_i8[r0 : r0 + P, 0:H2], in_=o8[:, 0:H2]
                    )
                    nc.vector.tensor_mul(xf[:, H2:], res[:, H2:], w_b[:, H2:])
                    nc.vector.tensor_scalar_mul(o8[:, H2:], xf[:, H2:], rstd[:])
                    nc.gpsimd.dma_start(
                        out=out_i8[r0 : r0 + P, H2:], in_=o8[:, H2:]
                    )

    nc.compile()
    return nc


def kernel(residual, x, weight, a):
    global last_results
    residual = np.ascontiguousarray(residual, dtype=np.float32)
    x = np.ascontiguousarray(x, dtype=np.int32)
    weight = np.ascontiguousarray(weight, dtype=np.float32)
    a_f = float(np.asarray(a))

    if x.min() >= -32768 and x.max() <= 32767:
        x_send = x.astype(np.int16)
        key = (a_f, "i16")
        x_dtype = mybir.dt.int16
    else:
        x_send = x
        key = (a_f, "i32")
        x_dtype = mybir.dt.int32

    if key not in _cache:
        _cache[key] = _build(a_f, x_dtype)
    nc = _cache[key]

    in_maps = [
        {
            "residual": residual[c * ROWS : (c + 1) * ROWS],
            "x": x_send[c * ROWS : (c + 1) * ROWS],
            "weight": weight,
        }
        for c in range(N_CORES)
    ]
    trace = os.environ.get("BASS_KERNEL_TRACE") == "1"
    try:
        last_results = run_bass_kernel_spmd(
            nc, in_maps, list(range(N_CORES)), trace=trace
        )
    except Exception:
        # transient device flakes (e.g. NRT_EXEC_UNIT_UNRECOVERABLE) have been
        # observed once on a cold NEFF; a single retry recovers
        last_results = run_bass_kernel_spmd(
            nc, in_maps, list(range(N_CORES)), trace=trace
        )
    res = last_results.results
    res_new = np.concatenate([res[c]["res_new"] for c in range(N_CORES)], axis=0)
    out_i8 = np.concatenate([res[c]["out_i8"] for c in range(N_CORES)], axis=0)
    return res_new, out_i8

